# revision 6
# baseline (speedup 1.0000x reference)
"""Trainium2 Bass kernel for nn_Canvas_DIP_by_distance (vq_codebook), v5.

reference semantics:
  weight = sigmoid(weight_logits)                       (224, 224, 3)
  d[h,w,c] = sum_k (palette[c,k] - weight[h,w,k])^2     (224, 224, 64)
  idx = argmax_c softmax(d + 1) = argmax_c d
  colors[ch,h,w] = palette[idx[h,w], ch]                (3, 224, 224)
  out = nearest_upsample(colors, 2048, 2048)            (3, 2048, 2048)

v5 design (per core: 28 canvas rows -> 256 output rows):
  - distances via ONE f16 matmul per (quarter, wf) using split precision:
    lhsT stacks [w16; dw; w16] (84 rows) against rhs [-p16|b3_16; -p16|*;
    -dp16|db3], so v = b3 - (w16 p16 + dw p16 + w16 dp) in fp32 PSUM.
    Host-verified: argmax identical to the fp32 reference on this data
    (max |v - v_fp32| ~ 6e-7, no fp32 ties). 3x cheaper than fp32
    LOW_HIGH double-pass matmuls on the (cold, 1.2 GHz) PE.
  - argmax one-hot + palette apply per quarter exactly as v2 (8x8
    factorization, fp32 is_equal on PSUM - tie-exact).
  - per half: colors tripled x3 into trip layout (one cheap DVE
    broadcast copy per wf), column expansion at [126, 512] x 4 (single
    wf each - cols < 1024 only ever read w < 112), fp32 copies into a
    per-half expT [126, 2048] buffer.
  - stores: per (quarter, ch) ONE broadcast dma (21 consecutive source
    partitions x 3-replica stride-0 axis -> rows 64g+1..64g+63). 21
    partitions -> 11 SDMA engines (vs 7 in the x9-broadcast variant).
    Rows 64g via 6 small tail DMAs.

trip layout: partition t = 63*gg + 21*ch + 3*j + c (gg = quarter within
half, c = one of 3 PE-made copies); canvas row hh = 7g + j.
"""

import numpy as np
from contextlib import ExitStack

CANVAS_H, CANVAS_W, NUM_COLORS = 224, 224, 64
IMAGE_H = IMAGE_W = 2048
N_CORES = 8
HC = CANVAS_H // N_CORES          # 28 canvas rows per core
ORC = IMAGE_H // N_CORES          # 256 output rows per core
WH = CANVAS_W // 2                # 112

_CACHE = {}


def _build_program():
    import concourse.bacc as bacc
    import concourse.tile as tile
    import concourse.mybir as mybir
    from concourse import bass

    f32 = mybir.dt.float32
    f16 = mybir.dt.float16
    ALU = mybir.AluOpType
    nc = bacc.Bacc("TRN2", target_bir_lowering=False)

    wb_in = nc.dram_tensor("wb_in", [84, 1344], f16, kind="ExternalInput")
    pi_in = nc.dram_tensor("pi_in", [112, 280], f16, kind="ExternalInput")
    esb_in = nc.dram_tensor("esb_in", [112, 2, 1024], f16, kind="ExternalInput")
    out = nc.dram_tensor("out", [3, ORC, IMAGE_W], f32, kind="ExternalOutput")

    with tile.TileContext(nc) as tc:
        with ExitStack() as ctx:
            sb = ctx.enter_context(tc.tile_pool(name="sb", bufs=1))
            ps = ctx.enter_context(tc.tile_pool(name="ps", bufs=1, space="PSUM"))

            # ---- const loads on the scalar ring (sync ring = stores only)
            wb = sb.tile([84, 1344], f16, tag="wb")
            nc.scalar.dma_start(out=wb[:], in_=wb_in[:])
            pi = sb.tile([112, 280], f16, tag="pi")
            nc.scalar.dma_start(out=pi[:], in_=pi_in[:])
            esb = sb.tile([112, 2, 1024], f16, tag="esb")
            nc.scalar.dma_start(out=esb[:], in_=esb_in[:])

            w4g3 = wb[:, 0:896].rearrange("p (f q x) -> p f q x", f=2, q=4)
            b4c3 = wb[:, 896:1344]
            p2e = pi[0:56, 0:168]
            id16 = pi[:, 168:280]

            colors = sb.tile([112, 2, 112], f16, tag="colors")
            nc.gpsimd.memset(colors[:], 0.0)

            # PSUM budget (8 banks): vps 3 + tps 1 + m1 1 + eps 2 = 7
            def front(q):
                """quarter q (canvas rows 7q..7q+6) -> colors slots."""
                m8a = sb.tile([112, 2, 7, 8], f32, tag="m8a", bufs=2)
                m8b = sb.tile([112, 2, 7, 8], f32, tag="m8b", bufs=2)
                for wf in range(2):
                    vq = ps.tile([112, 448], f32, tag="vps", bufs=3)
                    nc.tensor.matmul(
                        out=vq[:], lhsT=w4g3[:, wf, q],
                        rhs=b4c3, start=True, stop=True)
                    nc.vector.tensor_reduce(
                        out=m8a[:, wf],
                        in_=vq[:].rearrange("w (j a b) -> w j a b", a=8, b=8),
                        axis=mybir.AxisListType.X, op=ALU.max)
                    nc.vector.tensor_reduce(
                        out=m8b[:, wf],
                        in_=vq[:].rearrange("w (j a b) -> w j b a", a=8, b=8),
                        axis=mybir.AxisListType.X, op=ALU.max)
                vmax = sb.tile([112, 2, 7], f32, tag="vmax", bufs=2)
                nc.vector.tensor_reduce(
                    out=vmax[:], in_=m8a[:], axis=mybir.AxisListType.X,
                    op=ALU.max)
                vmb = vmax[:].unsqueeze(3).to_broadcast([112, 2, 7, 8])
                oha = sb.tile([112, 2, 7, 8], f16, tag="oha", bufs=2)
                nc.vector.tensor_tensor(
                    out=oha[:], in0=m8a[:], in1=vmb, op=ALU.is_equal)
                ohb = sb.tile([112, 2, 7, 8], f16, tag="ohb", bufs=2)
                nc.vector.tensor_tensor(
                    out=ohb[:], in0=m8b[:], in1=vmb, op=ALU.is_equal)
                m1 = ps.tile([112, 2, 256], f32, tag="m1ps", bufs=1)
                for wf in range(2):
                    tps = ps.tile([56, 112], f16, tag="tps", bufs=1)
                    nc.tensor.transpose(
                        out=tps[:],
                        in_=oha[:, wf].rearrange("w j a -> w (j a)"),
                        identity=id16)
                    ohaT = sb.tile([56, 112], f16, tag="ohaT", bufs=2)
                    nc.scalar.copy(out=ohaT[:], in_=tps[:])
                    nc.tensor.matmul(
                        out=m1[:, wf, 0:168], lhsT=ohaT[:], rhs=p2e,
                        start=True, stop=True)
                tmp = sb.tile([112, 2, 7, 3, 8], f16, tag="tmp", bufs=2)
                nc.vector.tensor_tensor(
                    out=tmp[:],
                    in0=m1[:, :, 0:168].rearrange(
                        "w f (j c b) -> w f j c b", c=3, b=8),
                    in1=ohb[:].unsqueeze(3).to_broadcast([112, 2, 7, 3, 8]),
                    op=ALU.mult)
                cdst = (colors[:, :, 28 * q:28 * q + 28]
                        .rearrange("w f (j s) -> w f j s", s=4)[:, :, :, 0:3])
                with nc.allow_low_precision(
                        reason="one-hot select: sum has a single nonzero f16"):
                    nc.vector.tensor_reduce(
                        out=cdst, in_=tmp[:], axis=mybir.AxisListType.X,
                        op=ALU.add)

            def half(h):
                """expand + store output rows 128h .. 128h+128."""
                c3 = sb.tile([112, 2, 126], f16, tag="c3", bufs=2)
                for wf in range(2):
                    src = (colors[:, wf, 56 * h:56 * h + 56]
                           .rearrange("w (g j s) -> w g s j", g=2, s=4)
                           [:, :, 0:3]
                           .unsqueeze(4).to_broadcast([112, 2, 3, 7, 3]))
                    dst = c3[:, wf].rearrange(
                        "w (g x j c) -> w g x j c", g=2, x=3, j=7)
                    nc.vector.tensor_copy(out=dst, in_=src)
                expT = sb.tile([126, 2048], f32, tag="expT", bufs=2)
                for cc in range(4):
                    wf = cc // 2
                    eps = ps.tile([126, 512], f32, tag="eps", bufs=2)
                    nc.tensor.matmul(
                        out=eps[:], lhsT=c3[:, wf],
                        rhs=esb[:, wf, 512 * (cc % 2):512 * (cc % 2) + 512],
                        start=True, stop=True)
                    nc.scalar.copy(
                        out=expT[:, 512 * cc:512 * cc + 512], in_=eps[:])
                for gg in range(2):
                    g = 2 * h + gg
                    for ch in range(3):
                        base = 63 * gg + 21 * ch
                        src = (expT[base:base + 21, :]
                               .unsqueeze(1).to_broadcast([21, 3, 2048]))
                        dst = (out[ch, 64 * g + 1:64 * g + 64, :]
                               .rearrange("(m r) c -> m r c", r=3))
                        nc.sync.dma_start(out=dst, in_=src)
                for ch in range(3):
                    # rows {128h, 128h+64} <- partitions {21ch, 21ch+63}
                    src = expT[21 * ch:21 * ch + 64:63, :]
                    dst = (out[ch, :, :]
                           .rearrange("(b a r) c -> b a r c",
                                      b=2, a=2, r=64)[h, :, 0])
                    nc.sync.dma_start(out=dst, in_=src)

            front(0)
            front(1)
            half(0)
            front(2)
            front(3)
            half(1)

    nc.compile()
    return nc


def _host_consts(weight_logits: np.ndarray, palette: np.ndarray):
    """Build per-core input tensors (host does sigmoid + layouts)."""
    pal32 = palette.astype(np.float32)
    pal16 = pal32.astype(np.float16)
    dp16 = (pal32 - pal16.astype(np.float32)).astype(np.float16)
    sig = (1.0 / (1.0 + np.exp(-weight_logits.astype(np.float64))))
    sig = sig.astype(np.float32)                      # (224, 224, 3)
    sig16 = sig.astype(np.float16)
    dsig = (sig - sig16.astype(np.float32)).astype(np.float16)

    b3_32 = 0.5 * (pal32.astype(np.float64) ** 2).sum(-1).astype(np.float32)
    b3_16 = b3_32.astype(np.float16)
    db3 = (b3_32 - b3_16.astype(np.float32)).astype(np.float16)

    # b4c3 [84 = 3 blocks x (7j 4k), 448 = (7j 64c)] block-diagonal f16
    def b4_block(prow, krow3):
        b4row = np.zeros((4, NUM_COLORS), np.float16)
        b4row[0:3] = -prow.T
        b4row[3] = krow3
        blk = np.zeros((28, 448), np.float16)
        for j in range(7):
            blk[4 * j:4 * j + 4, 64 * j:64 * j + 64] = b4row
        return blk
    b4c3 = np.concatenate([
        b4_block(pal16, b3_16),
        b4_block(pal16, b3_16),       # paired lhsT rows are dw (k=3 row = 0)
        b4_block(dp16, db3),
    ], axis=0)                                        # (84, 448)

    # p2e [56=(7j 8a), 168=(7j 3ch 8b)] block-diagonal
    p2 = pal16.reshape(8, 8, 3)                       # [a, b, ch]
    blk = np.transpose(p2, (0, 2, 1)).reshape(8, 24)  # [a, (ch b)]
    p2e = np.zeros((56, 168), np.float16)
    for j in range(7):
        p2e[8 * j:8 * j + 8, 24 * j:24 * j + 24] = blk

    # pi [112, 280]: p2e (padded to 112 rows) | id16
    pi = np.zeros((112, 280), np.float16)
    pi[0:56, 0:168] = p2e
    pi[:, 168:280] = np.eye(112, dtype=np.float16)

    # esb [112, 2, 1024]: wf-split 0/1 column-expansion
    wmap = (np.arange(IMAGE_W) * CANVAS_W) // IMAGE_W
    e_full = (wmap[None, :] == np.arange(CANVAS_W)[:, None]).astype(np.float16)
    esb = np.ascontiguousarray(
        np.stack([e_full[:WH, 0:1024], e_full[WH:, 1024:2048]], axis=1))

    # per-core wb [84, 1344]: w4g3 (2wf 4q 112) | b4c3
    wbs = []
    for core in range(N_CORES):
        s16 = sig16[core * HC:(core + 1) * HC]        # (28, 224, 3) f16
        ds = dsig[core * HC:(core + 1) * HC]
        w4g3 = np.zeros((84, 2, 4, 112), np.float16)
        for q in range(4):
            for j in range(7):
                r16 = s16[7 * q + j]                  # (224, 3)
                rds = ds[7 * q + j]
                for k in range(4):
                    if k < 3:
                        v1, v2, v3 = r16[:, k], rds[:, k], r16[:, k]
                    else:
                        one = np.ones(224, np.float16)
                        v1, v2, v3 = one, np.zeros(224, np.float16), one
                    for blk_i, v in enumerate((v1, v2, v3)):
                        w4g3[28 * blk_i + 4 * j + k, 0, q] = v[:WH]
                        w4g3[28 * blk_i + 4 * j + k, 1, q] = v[WH:]
        wb = np.concatenate(
            [w4g3.reshape(84, 896), b4c3], axis=1)    # (84, 1344)
        wbs.append(np.ascontiguousarray(wb))

    return wbs, pi, esb


def make_in_maps(weight_logits, palette):
    wbs, pi, esb = _host_consts(weight_logits, palette)
    return [{"wb_in": wbs[core], "pi_in": pi, "esb_in": esb}
            for core in range(N_CORES)]


def kernel(weight_logits, palette, image_h, image_w):
    weight_logits = np.asarray(weight_logits, np.float32)
    palette = np.asarray(palette, np.float32)
    assert int(image_h) == IMAGE_H and int(image_w) == IMAGE_W
    assert weight_logits.shape == (CANVAS_H, CANVAS_W, 3)

    if "nc" not in _CACHE:
        _CACHE["nc"] = _build_program()
    nc = _CACHE["nc"]

    from concourse import bass_utils

    res = bass_utils.run_bass_kernel_spmd(
        nc, make_in_maps(weight_logits, palette),
        core_ids=list(range(N_CORES)))
    outs = [res.results[c]["out"] for c in range(N_CORES)]
    return np.concatenate(outs, axis=1)


# revision 9
# speedup vs baseline: 1.5224x; 1.5224x over previous
"""Trainium2 Bass kernel for nn_Canvas_DIP_by_distance (vq_codebook), v5.

reference semantics:
  weight = sigmoid(weight_logits)                       (224, 224, 3)
  d[h,w,c] = sum_k (palette[c,k] - weight[h,w,k])^2     (224, 224, 64)
  idx = argmax_c softmax(d + 1) = argmax_c d
  colors[ch,h,w] = palette[idx[h,w], ch]                (3, 224, 224)
  out = nearest_upsample(colors, 2048, 2048)            (3, 2048, 2048)

v5 design (per core: 28 canvas rows -> 256 output rows):
  - distances via ONE f16 matmul per (quarter, wf) using split precision:
    lhsT stacks [w16; dw; w16] (84 rows) against rhs [-p16|b3_16; -p16|*;
    -dp16|db3], so v = b3 - (w16 p16 + dw p16 + w16 dp) in fp32 PSUM.
    Host-verified: argmax identical to the fp32 reference on this data
    (max |v - v_fp32| ~ 6e-7, no fp32 ties). 3x cheaper than fp32
    LOW_HIGH double-pass matmuls on the (cold, 1.2 GHz) PE.
  - argmax one-hot + palette apply per quarter exactly as v2 (8x8
    factorization, fp32 is_equal on PSUM - tie-exact).
  - per half: row-replicated colors cR[w, wf, ch, outrow] built by
    gpsimd broadcast-copies (free-dim stride-0 reads), then each output
    [128, 512] tile is ONE fused expand+replicate matmul (lhsT = cR,
    rhs = 0/1 esb; cols < 1024 only ever read w < 112 so each chunk
    needs a single wf). PSUM -> SBUF copies on ACT (DVE every 6th),
    stores as 12 x [128, 1024] 2-dim DMAs on the sync ring (full
    16-SDMA-engine spread; broadcast-source stores measured 7-engine /
    descriptor-generation bound and were abandoned).
"""

import numpy as np
from contextlib import ExitStack

CANVAS_H, CANVAS_W, NUM_COLORS = 224, 224, 64
IMAGE_H = IMAGE_W = 2048
N_CORES = 8
HC = CANVAS_H // N_CORES          # 28 canvas rows per core
ORC = IMAGE_H // N_CORES          # 256 output rows per core
WH = CANVAS_W // 2                # 112

_CACHE = {}


def _build_program():
    import concourse.bacc as bacc
    import concourse.tile as tile
    import concourse.mybir as mybir
    from concourse import bass

    f32 = mybir.dt.float32
    f16 = mybir.dt.float16
    ALU = mybir.AluOpType
    nc = bacc.Bacc("TRN2", target_bir_lowering=False)

    wb_in = nc.dram_tensor("wb_in", [84, 1344], f16, kind="ExternalInput")
    pi_in = nc.dram_tensor("pi_in", [112, 280], f16, kind="ExternalInput")
    esb_in = nc.dram_tensor("esb_in", [112, 2, 1024], f16, kind="ExternalInput")
    out = nc.dram_tensor("out", [3, ORC, IMAGE_W], f32, kind="ExternalOutput")

    with tile.TileContext(nc) as tc:
        with ExitStack() as ctx:
            sb = ctx.enter_context(tc.tile_pool(name="sb", bufs=1))
            ps = ctx.enter_context(tc.tile_pool(name="ps", bufs=1, space="PSUM"))

            # ---- const loads on the scalar ring (sync ring = stores only)
            wb = sb.tile([84, 1344], f16, tag="wb")
            nc.scalar.dma_start(out=wb[:], in_=wb_in[:])
            pi = sb.tile([112, 280], f16, tag="pi")
            nc.scalar.dma_start(out=pi[:], in_=pi_in[:])
            esb = sb.tile([112, 2, 1024], f16, tag="esb")
            nc.scalar.dma_start(out=esb[:], in_=esb_in[:])

            w4g3 = wb[:, 0:896].rearrange("p (f q x) -> p f q x", f=2, q=4)
            b4c3 = wb[:, 896:1344]
            p2e = pi[0:56, 0:168]
            id16 = pi[:, 168:280]

            colors = sb.tile([112, 2, 112], f16, tag="colors")
            nc.gpsimd.memset(colors[:], 0.0)

            ofs = sb.tile([128, 12, 1024], f32, tag="ofs")

            # PSUM budget (8 banks): vps 3 + tps 1 + m1 1 + ops 3 = 8
            def front(q):
                """quarter q (canvas rows 7q..7q+6) -> colors slots."""
                m8a = sb.tile([112, 2, 7, 8], f32, tag="m8a", bufs=2)
                m8b = sb.tile([112, 2, 7, 8], f32, tag="m8b", bufs=2)
                for wf in range(2):
                    vq = ps.tile([112, 448], f32, tag="vps", bufs=3)
                    nc.tensor.matmul(
                        out=vq[:], lhsT=w4g3[:, wf, q],
                        rhs=b4c3, start=True, stop=True)
                    nc.vector.tensor_reduce(
                        out=m8a[:, wf],
                        in_=vq[:].rearrange("w (j a b) -> w j a b", a=8, b=8),
                        axis=mybir.AxisListType.X, op=ALU.max)
                    nc.vector.tensor_reduce(
                        out=m8b[:, wf],
                        in_=vq[:].rearrange("w (j a b) -> w j b a", a=8, b=8),
                        axis=mybir.AxisListType.X, op=ALU.max)
                vmax = sb.tile([112, 2, 7], f32, tag="vmax", bufs=2)
                nc.vector.tensor_reduce(
                    out=vmax[:], in_=m8a[:], axis=mybir.AxisListType.X,
                    op=ALU.max)
                vmb = vmax[:].unsqueeze(3).to_broadcast([112, 2, 7, 8])
                oha = sb.tile([112, 2, 7, 8], f16, tag="oha", bufs=2)
                nc.vector.tensor_tensor(
                    out=oha[:], in0=m8a[:], in1=vmb, op=ALU.is_equal)
                ohb = sb.tile([112, 2, 7, 8], f16, tag="ohb", bufs=2)
                nc.vector.tensor_tensor(
                    out=ohb[:], in0=m8b[:], in1=vmb, op=ALU.is_equal)
                m1 = ps.tile([112, 2, 256], f32, tag="m1ps", bufs=1)
                for wf in range(2):
                    tps = ps.tile([56, 112], f16, tag="tps", bufs=1)
                    nc.tensor.transpose(
                        out=tps[:],
                        in_=oha[:, wf].rearrange("w j a -> w (j a)"),
                        identity=id16)
                    ohaT = sb.tile([56, 112], f16, tag="ohaT", bufs=2)
                    nc.scalar.copy(out=ohaT[:], in_=tps[:])
                    nc.tensor.matmul(
                        out=m1[:, wf, 0:168], lhsT=ohaT[:], rhs=p2e,
                        start=True, stop=True)
                tmp = sb.tile([112, 2, 7, 3, 8], f16, tag="tmp", bufs=2)
                nc.vector.tensor_tensor(
                    out=tmp[:],
                    in0=m1[:, :, 0:168].rearrange(
                        "w f (j c b) -> w f j c b", c=3, b=8),
                    in1=ohb[:].unsqueeze(3).to_broadcast([112, 2, 7, 3, 8]),
                    op=ALU.mult)
                cdst = (colors[:, :, 28 * q:28 * q + 28]
                        .rearrange("w f (j s) -> w f j s", s=4)[:, :, :, 0:3])
                with nc.allow_low_precision(
                        reason="one-hot select: sum has a single nonzero f16"):
                    nc.vector.tensor_reduce(
                        out=cdst, in_=tmp[:], axis=mybir.AxisListType.X,
                        op=ALU.add)

            cnt = [0]

            def half(h):
                """fused expand+replicate + store rows 128h .. 128h+128.

                cR[w, wf, ch, 64g'+m] = colors[w, wf, slot(hh(128h+64g'+m),
                ch)] built by gpsimd broadcast-copies; then each output
                [128, 512] tile is ONE matmul lhsT=cR, rhs=esb."""
                cR = sb.tile([112, 2, 3, 128], f16, tag="cR", bufs=2)
                for wf in range(2):
                    for ch in range(3):
                        src9 = (colors[:, wf, 56 * h:56 * h + 56]
                                .rearrange("w (g j s) -> w g j s", g=2, s=4)
                                [:, :, :, ch]
                                .unsqueeze(3).to_broadcast([112, 2, 7, 9]))
                        dst9 = (cR[:, wf, ch]
                                .rearrange("w (g r) -> w g r", g=2)
                                [:, :, 1:64]
                                .rearrange("w g (j r) -> w g j r", j=7))
                        nc.gpsimd.tensor_copy(out=dst9, in_=src9)
                        nc.gpsimd.tensor_copy(
                            out=cR[:, wf, ch, 0:65:64],
                            in_=colors[:, wf,
                                       56 * h + ch:56 * h + ch + 29:28])
                for ch in range(3):
                    for p in range(2):
                        islot = 6 * h + 2 * ch + p
                        for sub in range(2):
                            cc = 2 * p + sub
                            wf = cc // 2
                            ops = ps.tile([128, 512], f32, tag="ops", bufs=3)
                            nc.tensor.matmul(
                                out=ops[:], lhsT=cR[:, wf, ch],
                                rhs=esb[:, wf,
                                        512 * (cc % 2):512 * (cc % 2) + 512],
                                start=True, stop=True)
                            oslice = ofs[:, islot, 512 * sub:512 * sub + 512]
                            cnt[0] += 1
                            if cnt[0] % 6 == 0:
                                nc.vector.tensor_copy(out=oslice, in_=ops[:])
                            else:
                                nc.scalar.copy(out=oslice, in_=ops[:])
                        nc.sync.dma_start(
                            out=out[ch, 128 * h:128 * h + 128,
                                    1024 * p:1024 * p + 1024],
                            in_=ofs[:, islot, :])

            front(0)
            front(1)
            half(0)
            front(2)
            front(3)
            half(1)

    nc.compile()
    return nc


def _host_consts(weight_logits: np.ndarray, palette: np.ndarray):
    """Build per-core input tensors (host does sigmoid + layouts)."""
    pal32 = palette.astype(np.float32)
    pal16 = pal32.astype(np.float16)
    dp16 = (pal32 - pal16.astype(np.float32)).astype(np.float16)
    sig = (1.0 / (1.0 + np.exp(-weight_logits.astype(np.float64))))
    sig = sig.astype(np.float32)                      # (224, 224, 3)
    sig16 = sig.astype(np.float16)
    dsig = (sig - sig16.astype(np.float32)).astype(np.float16)

    b3_32 = 0.5 * (pal32.astype(np.float64) ** 2).sum(-1).astype(np.float32)
    b3_16 = b3_32.astype(np.float16)
    db3 = (b3_32 - b3_16.astype(np.float32)).astype(np.float16)

    # b4c3 [84 = 3 blocks x (7j 4k), 448 = (7j 64c)] block-diagonal f16
    def b4_block(prow, krow3):
        b4row = np.zeros((4, NUM_COLORS), np.float16)
        b4row[0:3] = -prow.T
        b4row[3] = krow3
        blk = np.zeros((28, 448), np.float16)
        for j in range(7):
            blk[4 * j:4 * j + 4, 64 * j:64 * j + 64] = b4row
        return blk
    b4c3 = np.concatenate([
        b4_block(pal16, b3_16),
        b4_block(pal16, b3_16),       # paired lhsT rows are dw (k=3 row = 0)
        b4_block(dp16, db3),
    ], axis=0)                                        # (84, 448)

    # p2e [56=(7j 8a), 168=(7j 3ch 8b)] block-diagonal
    p2 = pal16.reshape(8, 8, 3)                       # [a, b, ch]
    blk = np.transpose(p2, (0, 2, 1)).reshape(8, 24)  # [a, (ch b)]
    p2e = np.zeros((56, 168), np.float16)
    for j in range(7):
        p2e[8 * j:8 * j + 8, 24 * j:24 * j + 24] = blk

    # pi [112, 280]: p2e (padded to 112 rows) | id16
    pi = np.zeros((112, 280), np.float16)
    pi[0:56, 0:168] = p2e
    pi[:, 168:280] = np.eye(112, dtype=np.float16)

    # esb [112, 2, 1024]: wf-split 0/1 column-expansion
    wmap = (np.arange(IMAGE_W) * CANVAS_W) // IMAGE_W
    e_full = (wmap[None, :] == np.arange(CANVAS_W)[:, None]).astype(np.float16)
    esb = np.ascontiguousarray(
        np.stack([e_full[:WH, 0:1024], e_full[WH:, 1024:2048]], axis=1))

    # per-core wb [84, 1344]: w4g3 (2wf 4q 112) | b4c3
    wbs = []
    for core in range(N_CORES):
        s16 = sig16[core * HC:(core + 1) * HC]        # (28, 224, 3) f16
        ds = dsig[core * HC:(core + 1) * HC]
        w4g3 = np.zeros((84, 2, 4, 112), np.float16)
        for q in range(4):
            for j in range(7):
                r16 = s16[7 * q + j]                  # (224, 3)
                rds = ds[7 * q + j]
                for k in range(4):
                    if k < 3:
                        v1, v2, v3 = r16[:, k], rds[:, k], r16[:, k]
                    else:
                        one = np.ones(224, np.float16)
                        v1, v2, v3 = one, np.zeros(224, np.float16), one
                    for blk_i, v in enumerate((v1, v2, v3)):
                        w4g3[28 * blk_i + 4 * j + k, 0, q] = v[:WH]
                        w4g3[28 * blk_i + 4 * j + k, 1, q] = v[WH:]
        wb = np.concatenate(
            [w4g3.reshape(84, 896), b4c3], axis=1)    # (84, 1344)
        wbs.append(np.ascontiguousarray(wb))

    return wbs, pi, esb


def make_in_maps(weight_logits, palette):
    wbs, pi, esb = _host_consts(weight_logits, palette)
    return [{"wb_in": wbs[core], "pi_in": pi, "esb_in": esb}
            for core in range(N_CORES)]


def kernel(weight_logits, palette, image_h, image_w):
    weight_logits = np.asarray(weight_logits, np.float32)
    palette = np.asarray(palette, np.float32)
    assert int(image_h) == IMAGE_H and int(image_w) == IMAGE_W
    assert weight_logits.shape == (CANVAS_H, CANVAS_W, 3)

    if "nc" not in _CACHE:
        _CACHE["nc"] = _build_program()
    nc = _CACHE["nc"]

    from concourse import bass_utils

    res = bass_utils.run_bass_kernel_spmd(
        nc, make_in_maps(weight_logits, palette),
        core_ids=list(range(N_CORES)))
    outs = [res.results[c]["out"] for c in range(N_CORES)]
    return np.concatenate(outs, axis=1)
